# revision 16
# baseline (speedup 1.0000x reference)
"""GQA attention block (B=2,T=2048,E=2048,H=16,KV=4) on 8 trn2 NeuronCores.

Sharding: core c -> batch b=c//4, kv-group g=c%4 (q-heads 4g..4g+3, kv head g).
Each core computes its 4 heads end-to-end plus the partial output projection
(Wo rows for its heads); host sums the 4 partials per batch and adds bias.

Device-side layout tricks (all cores run one identical program, data differs):
  - q/k are produced directly transposed [d, T] (d on partitions) so the
    scores matmul S^T = kT.T-block @ qT and the PV matmul need no transposes.
  - RoPE pair interleave removed by host-permuting Wq/Wk columns per head to
    [64 even | 64 odd]; the r/i half swap is done with cross-partition-base
    DVE multiplies (no SBUF-SBUF DMA).
  - rope(q + bq) handled by adding bq (per-partition scalar) during PSUM
    evacuation, before the cos/sin multiply. bv folded into bo on host
    (attn rows sum to const after softmax). Softmax post-scale folded into Wo.
  - Softmax over the partition dim (S^T rows) via ones-vector matmuls on PE,
    reciprocal on DVE, and a Pool-engine partition broadcast for the divide.
  - Causal diagonal tiles trim their matmul free-range to the unmasked
    queries (~25% less scores/sum/PV work).
  - DMAs are issued in consumption order (wq/xt chunk 0 interleaved first)
    so the Q projection starts ~1us in instead of waiting for all of x.
"""

import numpy as np

for _p in ("/opt/trn_rl_repo", "/root/.axon_site/_ro/trn_rl_repo"):
    import sys

    if _p not in sys.path:
        sys.path.insert(0, _p)

import ml_dtypes
from contextlib import ExitStack

import concourse.bass as bass
import concourse.mybir as mybir
import concourse.tile as tile
from concourse import bacc
from concourse.bass_utils import run_bass_kernel_spmd

F32 = mybir.dt.float32
BF16 = mybir.dt.bfloat16
F16 = mybir.dt.float16
T = 2048
E = 2048
HD = 128
NQH = 4          # q heads per core
SCALE = float(E) ** -0.5

_program = None
LAST_EXEC_NS = None


def _build_program():
    nc = bacc.Bacc("TRN2", target_bir_lowering=False, debug=False, num_devices=8)
    xT_d = nc.declare_dram_parameter("xT", [E, T], F16, isOutput=False)
    wq_d = nc.declare_dram_parameter("wq", [E, NQH * HD], F16, isOutput=False)
    wkv_d = nc.declare_dram_parameter("wkv", [E, 2 * HD], F16, isOutput=False)
    wo_d = nc.declare_dram_parameter("wo", [NQH * HD, E], BF16, isOutput=False)
    cst_d = nc.declare_dram_parameter("cst", [HD, 2 * T], F32, isOutput=False)
    mask_d = nc.declare_dram_parameter("mask", [HD, 4 * 512], BF16, isOutput=False)
    bq_d = nc.declare_dram_parameter("bq", [HD, NQH], F32, isOutput=False)
    bk_d = nc.declare_dram_parameter("bk", [HD, 1], F32, isOutput=False)
    out_d = nc.declare_dram_parameter("out", [T, E], F16, isOutput=True)

    KT = E // 128    # 16 k-tiles over embed
    NT = T // 128    # 16 tiles over time
    NC = T // 512    # 4 512-chunks over time

    with tile.TileContext(nc) as tc, ExitStack() as ctx:
        consts = ctx.enter_context(tc.tile_pool(name="consts", bufs=1))
        rope = ctx.enter_context(tc.tile_pool(name="rope", bufs=2))
        ptp = ctx.enter_context(tc.tile_pool(name="ptp", bufs=8))
        ptep = ctx.enter_context(tc.tile_pool(name="ptep", bufs=3))
        otp = ctx.enter_context(tc.tile_pool(name="otp", bufs=10))
        outp = ctx.enter_context(tc.tile_pool(name="outp", bufs=2))
        dvp = ctx.enter_context(tc.tile_pool(name="dvp", bufs=2))
        bip = ctx.enter_context(tc.tile_pool(name="bip", bufs=2))
        psA = ctx.enter_context(tc.tile_pool(name="psA", bufs=5, space=bass.MemorySpace.PSUM))
        psOT = ctx.enter_context(tc.tile_pool(name="psOT", bufs=2, space=bass.MemorySpace.PSUM))
        psD = ctx.enter_context(tc.tile_pool(name="psD", bufs=1, space=bass.MemorySpace.PSUM))

        # ---- resident constants, DMA'd in consumption order ---------------
        # Q proj (c=0) consumes wq[k] + xt[k][:, 0:512] pairs first.
        wq = [consts.tile([128, NQH * HD], F16, tag=f"wq{k}", name=f"wq{k}")
              for k in range(KT)]
        xt = [consts.tile([128, T], F16, tag=f"xt{k}", name=f"xt{k}")
              for k in range(KT)]
        for k in range(KT):
            nc.sync.dma_start(wq[k][:], wq_d[k * 128:(k + 1) * 128, :])
            nc.sync.dma_start(xt[k][:, 0:512], xT_d[k * 128:(k + 1) * 128, 0:512])
        bq_t = consts.tile([HD, NQH], F32, tag="bq")
        nc.sync.dma_start(bq_t[:], bq_d[:])
        bk_t = consts.tile([HD, 1], F32, tag="bk")
        nc.sync.dma_start(bk_t[:], bk_d[:])
        # cst holds [cos | sin] interleaved per 512-chunk: one DMA per chunk
        cst = consts.tile([128, 2 * T], F32, tag="cst")
        nc.sync.dma_start(cst[:, 0:1024], cst_d[:, 0:1024])
        wkv = [consts.tile([128, 2 * HD], F16, tag=f"wkv{k}", name=f"wkv{k}")
               for k in range(KT)]
        wk = [t_[:, 0:HD] for t_ in wkv]
        wv = [t_[:, HD:2 * HD] for t_ in wkv]
        for k in range(KT):
            nc.sync.dma_start(wkv[k][:], wkv_d[k * 128:(k + 1) * 128, :])
        def _xt_chunk(c, span=1):
            nc.sync.dma_start(cst[:, c * 1024:(c + span) * 1024],
                              cst_d[:, c * 1024:(c + span) * 1024])
            for k in range(KT):
                nc.sync.dma_start(xt[k][:, c * 512:(c + span) * 512],
                                  xT_d[k * 128:(k + 1) * 128,
                                       c * 512:(c + span) * 512])
        _xt_chunk(1)
        mskt = consts.tile([128, 4 * 512], BF16, tag="mskt")
        nc.sync.dma_start(mskt[:], mask_d[:])
        msk = [mskt[:, j * 512:(j + 1) * 512] for j in range(4)]
        _xt_chunk(2, span=2)
        wo = []
        for h in range(NQH):
            t_ = consts.tile([128, E], BF16, tag=f"wo{h}")
            nc.sync.dma_start(t_[:], wo_d[h * 128:(h + 1) * 128, :])
            wo.append(t_)
        ones_col = consts.tile([128, 1], BF16, tag="onc")
        nc.vector.memset(ones_col[:], 1.0)

        qT = []
        for h in range(NQH):
            qT.append(consts.tile([128, T], BF16, tag=f"qT{h}", name=f"qT{h}"))
        kTt = consts.tile([128, T], BF16, tag="kT")
        vA = consts.tile([128, T], BF16, tag="vA")

        # ---- projections + rope -------------------------------------------
        def rope_chunk(ps, bias_ap, dst, col0):
            cts = slice(2 * col0, 2 * col0 + 512)
            sts = slice(2 * col0 + 512, 2 * col0 + 1024)
            qsb = rope.tile([128, 512], F32, tag="qsb")
            nc.scalar.activation(
                qsb[:], ps[:], mybir.ActivationFunctionType.Identity, bias=bias_ap)
            t1 = rope.tile([128, 512], F32, tag="t1")
            nc.vector.tensor_mul(t1[:], qsb[:], cst[:, cts])
            # r/i half swap: engines can't cross partition bases (walrus
            # samePartitionsAll), so swap via 2 DMAs on the Pool queue
            # (doesn't HoL-block the SP input-load queue)
            qsw = rope.tile([128, 512], F32, tag="qsw")
            nc.gpsimd.dma_start(qsw[0:64, :], qsb[64:128, :])
            nc.gpsimd.dma_start(qsw[64:128, :], qsb[0:64, :])
            t2 = rope.tile([128, 512], F32, tag="t2")
            nc.vector.tensor_mul(t2[:], qsw[:], cst[:, sts])
            nc.vector.tensor_add(dst[:, sl(col0)], t1[:], t2[:])

        def sl(col0):
            return slice(col0, col0 + 512)

        # per chunk: Q (4 heads), K, V — all consume the same xt chunk, so
        # the PE has ~20us of work per 2.5MB of x DMA (which takes ~8us)
        for c in range(NC):
            for h in range(NQH):
                ps = psA.tile([128, 512], F32, tag="ps")
                for k in range(KT):
                    nc.tensor.matmul(
                        ps[:], wq[k][:, h * HD:(h + 1) * HD],
                        xt[k][:, c * 512:(c + 1) * 512],
                        start=(k == 0), stop=(k == KT - 1))
                rope_chunk(ps, bq_t[:, h:h + 1], qT[h], c * 512)
            ps = psA.tile([128, 512], F32, tag="ps")
            for k in range(KT):
                nc.tensor.matmul(
                    ps[:], wk[k], xt[k][:, c * 512:(c + 1) * 512],
                    start=(k == 0), stop=(k == KT - 1))
            rope_chunk(ps, bk_t[:, 0:1], kTt, c * 512)
            vps = psD.tile([128, 512], F32, tag="psd")
            for j in range(4):
                tt = 4 * c + j
                for k in range(KT):
                    nc.tensor.matmul(
                        vps[:, j * 128:(j + 1) * 128],
                        xt[k][:, tt * 128:(tt + 1) * 128], wv[k],
                        start=(k == 0), stop=(k == KT - 1))
            nc.scalar.copy(vA[:, c * 512:(c + 1) * 512], vps[:])

        # ---- attention + output projection, per 512-query chunk -----------
        # out-proj of chunk qc is emitted after the attention heads of
        # chunk qc+1, hiding the softmax-divide chain latency behind the
        # next chunk's score matmuls.
        def attn_heads(qc):
            ots = []
            for h in range(NQH):
                ntk = 4 * (qc + 1)
                psd = psD.tile([1, 512], F32, tag="psd")
                psot = psOT.tile([128, 512], F32, tag="psot")
                for tk in range(ntk):
                    # diagonal tiles: only queries >= tile's first key are
                    # unmasked; trim the matmul free-range to [off, 512)
                    off = max(0, (tk - 4 * qc) * 128)
                    w = 512 - off
                    pss = psA.tile([128, 512], F32, tag="ps")
                    nc.tensor.matmul(
                        pss[:, 0:w], kTt[:, tk * 128:(tk + 1) * 128],
                        qT[h][:, qc * 512 + off:(qc + 1) * 512],
                        start=True, stop=True)
                    pt = ptp.tile([128, 512], BF16, tag="pt")
                    if tk >= 4 * qc:
                        pte = ptep.tile([128, 512], BF16, tag="pte")
                        nc.scalar.activation(
                            pte[:, 0:w], pss[:, 0:w],
                            mybir.ActivationFunctionType.Exp)
                        nc.vector.tensor_mul(
                            pt[:, 0:w], pte[:, 0:w],
                            mskt[:, (tk - 4 * qc) * 512 + off:(tk - 4 * qc) * 512 + 512])
                    else:
                        nc.scalar.activation(
                            pt[:, 0:w], pss[:, 0:w],
                            mybir.ActivationFunctionType.Exp)
                    nc.tensor.matmul(
                        psd[0:1, off:512], ones_col[:], pt[:, 0:w],
                        start=(tk == 0), stop=(tk == ntk - 1))
                    nc.tensor.matmul(
                        psot[:, off:512], vA[:, tk * 128:(tk + 1) * 128],
                        pt[:, 0:w],
                        start=(tk == 0), stop=(tk == ntk - 1))
                dinv = dvp.tile([1, 512], F32, tag="dinv")
                nc.vector.reciprocal(dinv[:], psd[:])
                binv = bip.tile([128, 512], F32, tag="binv")
                nc.gpsimd.partition_broadcast(binv[:], dinv[:], channels=128)
                otn = otp.tile([128, 512], BF16, tag="otn")
                nc.vector.tensor_mul(otn[:], psot[:], binv[:])
                ots.append(otn)
            return ots

        def out_proj(qc, ots):
            for i in range(4):
                osb = outp.tile([128, E], F16, tag="osb")
                for e in range(NC):
                    psf = psA.tile([128, 512], F32, tag="ps")
                    for h in range(NQH):
                        nc.tensor.matmul(
                            psf[:], ots[h][:, i * 128:(i + 1) * 128],
                            wo[h][:, e * 512:(e + 1) * 512],
                            start=(h == 0), stop=(h == NQH - 1))
                    nc.scalar.copy(osb[:, e * 512:(e + 1) * 512], psf[:])
                nc.gpsimd.dma_start(
                    out_d[(qc * 4 + i) * 128:(qc * 4 + i + 1) * 128, :],
                    osb[:])

        prev = None
        for qc in range(NC):
            ots = attn_heads(qc)
            if prev is not None:
                out_proj(*prev)
            prev = (qc, ots)
        out_proj(*prev)
    nc.compile()
    return nc


def _rope_tables():
    # quirk: freq exponent uses full n_embed then slices to head_dim//2
    freqs = 10000.0 ** (-(np.arange(0, E, 2, dtype=np.float64) / E))[:HD // 2]
    t = np.arange(T, dtype=np.float64)
    ang = np.outer(freqs, t)                      # [64, T]
    ct = np.empty((HD, T), np.float32)
    st = np.empty((HD, T), np.float32)
    ct[:64] = np.cos(ang)
    ct[64:] = np.cos(ang)
    st[:64] = -np.sin(ang)
    st[64:] = np.sin(ang)
    return ct, st


def kernel(x, Wq, bq, Wk, bk, Wv, bv, Wo, bo):
    global _program, LAST_EXEC_NS
    x = np.asarray(x, np.float32)
    Wq, bq = np.asarray(Wq, np.float32), np.asarray(bq, np.float32)
    Wk, bk = np.asarray(Wk, np.float32), np.asarray(bk, np.float32)
    Wv, bv = np.asarray(Wv, np.float32), np.asarray(bv, np.float32)
    Wo, bo = np.asarray(Wo, np.float32), np.asarray(bo, np.float32)
    bf = ml_dtypes.bfloat16

    if _program is None:
        _program = _build_program()

    perm = np.concatenate([np.arange(0, HD, 2), np.arange(1, HD, 2)])
    ct, st = _rope_tables()
    # [cos_c | sin_c] per 512-chunk along the free dim
    cst_h = np.empty((HD, 2 * T), np.float32)
    for c in range(4):
        cst_h[:, c * 1024:c * 1024 + 512] = ct[:, c * 512:(c + 1) * 512]
        cst_h[:, c * 1024 + 512:(c + 1) * 1024] = st[:, c * 512:(c + 1) * 512]
    mask = np.zeros((4, HD, 512), np.float32)
    cc = np.arange(512)[None, :]
    rr = np.arange(HD)[:, None]
    for j in range(4):
        mask[j] = (cc >= HD * j + rr).astype(np.float32)
    mask_h = np.ascontiguousarray(
        mask.transpose(1, 0, 2).reshape(HD, 4 * 512)).astype(bf)

    xT = [np.ascontiguousarray(x[b].T).astype(np.float16) for b in range(2)]
    in_maps = []
    for c in range(8):
        b, g = divmod(c, 4)
        qcols = np.concatenate([(4 * g + h) * HD + perm for h in range(NQH)])
        kcols = g * HD + perm
        vcols = np.arange(g * HD, (g + 1) * HD)
        wkv = np.concatenate([Wk[:, kcols], Wv[:, vcols]], axis=1)
        in_maps.append({
            "xT": xT[b],
            "wq": Wq[:, qcols].astype(np.float16),
            "wkv": wkv.astype(np.float16),
            "wo": (Wo[g * 512:(g + 1) * 512, :] * SCALE).astype(bf),
            "cst": cst_h,
            "mask": mask_h,
            "bq": np.ascontiguousarray(
                bq[np.concatenate([(4 * g + h) * HD + perm for h in range(NQH)])]
                .reshape(NQH, HD).T).astype(np.float32),
            "bk": bk[kcols].reshape(HD, 1).astype(np.float32),
        })

    import time
    t0 = time.time()
    res = run_bass_kernel_spmd(_program, in_maps, list(range(8)))
    t1 = time.time()
    LAST_EXEC_NS = res.exec_time_ns
    if LAST_EXEC_NS is None:
        LAST_EXEC_NS = int((t1 - t0) * 1e9)  # wall time incl. H2D (upper bound)

    out = np.zeros((2, T, E), np.float64)
    for c in range(8):
        out[c // 4] += np.asarray(res.results[c]["out"], np.float64)
    # bv folded: after softmax each row sums to 1, scaled by SCALE inside Wo
    obias = np.repeat(bv.astype(np.float64).reshape(4, HD), 4, axis=0).reshape(-1)
    bo_eff = bo.astype(np.float64) + SCALE * (obias @ Wo.astype(np.float64))
    out += bo_eff[None, None, :]
    return out.astype(np.float32)
